# revision 26
# baseline (speedup 1.0000x reference)
"""Trainium2 Bass kernel for a single-layer RNN (tanh) + final linear.

Problem: B=64, T=512, I=256, H=1024, O=128 (fp32).
    xp = einsum('bti,hi->tbh', x, W_ih) + b_ih + b_hh
    h_t = tanh(xp_t + h_{t-1} @ W_hh.T)         (T sequential steps)
    y   = h_T @ W_lin.T + b_lin

Sharding: data-parallel over batch, 8 cores x 8 rows each. Each core runs
the full recurrence for its batch shard; no collectives.

Per-core scheme ("T-layout"):
  The recurrence matmul keeps h as the 128x8 stationary operand (batch=8
  output rows per PSUM col-group) and streams W_hh^T quarters through the
  four 32-wide column groups of the PE array concurrently. The PSUM result
  lands batch-major ([32j+b, n] = z[b, 256j+n]). Post chain per step is
  tanh FIRST (ACT reads PSUM directly — faster access — writes bf16 to
  SBUF), then a DVE 32x32 block transpose of the bf16 tanh output into
  T-layout: T[32J+r, 128h+32N+c] = h_new[c, 256J+128h+32N+r], so the
  stationary slice for recurrence k-chunk f=(4h+N) is the contiguous
  T[:, 128h+32N : 128h+32N+8]. The input projection x @ W_ih^T runs in
  bf16 (fp32 moving data costs 4 cycles/row on the PE — 2x LOW/HIGH
  instruction split at half rate) a few steps ahead into the same PSUM
  accumulation group, filling the PE during the tanh/transpose boundary.

All weight-layout permutations are precomputed host-side in numpy.

Post-compile, _strip_pe_ticks removes the per-matmul PE tick-semaphore
increments that no wait targets (keeping exact wait semantics): the hw
drains sem updates at ~34ns each, so the 32 recurrence matmuls per step
otherwise delay the tanh gate by ~190ns of update-queue backlog.
"""

import os
import sys

import ml_dtypes
import numpy as np

BF16 = ml_dtypes.bfloat16

for _p in ("/root/.axon_site", "/root/.axon_site/_ro/trn_rl_repo",
           "/root/.axon_site/_ro/pypackages", "/opt/trn_rl_repo"):
    if os.path.isdir(_p) and _p not in sys.path:
        sys.path.append(_p)

B, I, H, O = 64, 256, 1024, 128
NCORES = 8
B_LOC = B // NCORES  # 8
LOOKAHEAD = 1        # projection runs this many steps ahead of the recurrence

_module_cache = {}


def _strip_pe_ticks(nc):
    """Drop the per-matmul PE tick-sem increments that nothing targets.

    Every InstMatmult carries a +1 update on the PE tick semaphore, and the
    hardware drains these at ~34ns each — so the 32 recurrence matmuls of a
    step delay the tanh's semaphore gate by ~190ns beyond f7's completion
    (the update queue backlog).  Keep only the increments whose tick index
    is the exact target of some wait (plus the last), and remap every wait
    value to its rank among kept ticks.  Since the PE executes in order,
    each wait still releases at the completion of the same instruction as
    before — identical ordering semantics, ~40x less semaphore traffic.
    """
    import concourse.mybir as mybir

    f = nc.m.functions[0]
    insts = []
    for bb in f.blocks:
        insts.extend(bb.instructions)
    tickers = []
    for i in insts:
        if isinstance(i, mybir.InstMatmult) and i.sync_info:
            for u in i.sync_info.on_update:
                if u.update_mode == "sem-inc":
                    tickers.append((i, u.id))
    if not tickers:
        return
    semid = tickers[0][1]
    assert all(s == semid for _, s in tickers)
    waits = []
    for i in insts:
        si = i.sync_info
        if not si:
            continue
        for w in si.on_wait:
            if w.id == semid:
                assert w.wait_mode == "sem-ge-imm" and w.wait_reg is None
                waits.append(w)
    n = len(tickers)
    keep = {n - 1}
    for w in waits:
        assert 1 <= w.wait_value <= n, (w.wait_value, n)
        keep.add(w.wait_value - 1)
    rank = [0] * n
    c = 0
    for idx in range(n):
        if idx in keep:
            c += 1
        rank[idx] = c
    for w in waits:
        w.wait_value = rank[w.wait_value - 1]
    for idx, (i, _) in enumerate(tickers):
        if idx not in keep:
            i.sync_info.on_update = [
                u for u in i.sync_info.on_update
                if not (u.id == semid and u.update_mode == "sem-inc")]


def _build_module(t_steps, sim=False):
    """Trace + compile the Bass module for a given sequence length."""
    key = (t_steps, sim)
    if key in _module_cache:
        return _module_cache[key]

    from contextlib import ExitStack

    import concourse.bacc as bacc
    import concourse.mybir as mybir
    import concourse.tile as tile
    from concourse.tile_rust import add_dep_helper

    f32 = mybir.dt.float32
    bf16 = mybir.dt.bfloat16
    Tanh = mybir.ActivationFunctionType.Tanh

    nc = bacc.Bacc("TRN2", target_bir_lowering=False, debug=False,
                   enable_asserts=False)

    xT_d = nc.dram_tensor("xT", [128, 2 * t_steps * B_LOC], bf16,
                          kind="ExternalInput")
    wt_d = nc.dram_tensor("wt", [128, 8 * H], bf16, kind="ExternalInput")
    wih_d = nc.dram_tensor("wih", [128, 2 * H], bf16, kind="ExternalInput")
    wlin_d = nc.dram_tensor("wlin", [128, 8 * O], bf16, kind="ExternalInput")
    bias_d = nc.dram_tensor("bias1", [1, H], bf16, kind="ExternalInput")
    y_d = nc.dram_tensor("y", [B_LOC, O], f32, kind="ExternalOutput")

    with tile.TileContext(nc) as tc, ExitStack() as ctx:
        wpool = ctx.enter_context(tc.tile_pool(name="weights", bufs=1))
        ppool = ctx.enter_context(tc.tile_pool(name="psum", bufs=LOOKAHEAD + 2,
                                               space="PSUM"))
        tpool = ctx.enter_context(tc.tile_pool(name="tbuf", bufs=2))

        # Small inputs first (proj(0) gates on them), then the 2 MB wt
        # (needed ~2 steps later), then the rest of x lazily; xT is t-major
        # ((t, k, b) columns) and DMA'd in chunks so proj(0) gates on the
        # first 32 steps of x, not the whole 2 MB.
        wih_sb = wpool.tile([128, 2 * H], bf16, name="wih_sb")
        nc.sync.dma_start(out=wih_sb, in_=wih_d.ap())
        bias_sb = wpool.tile([1, H], bf16, name="bias_sb")
        nc.sync.dma_start(out=bias_sb, in_=bias_d.ap())
        xT_sb = wpool.tile([128, 2 * t_steps * B_LOC], bf16, name="xT_sb")
        CHUNK = 32 * 2 * B_LOC
        nc.sync.dma_start(out=xT_sb[:, 0:CHUNK], in_=xT_d.ap()[:, 0:CHUNK])
        wt_sb = wpool.tile([128, 8 * H], bf16, name="wt_sb")
        nc.sync.dma_start(out=wt_sb, in_=wt_d.ap())
        wlin_sb = wpool.tile([128, 8 * O], bf16, name="wlin_sb")
        nc.sync.dma_start(out=wlin_sb, in_=wlin_d.ap())
        for c in range(CHUNK, 2 * t_steps * B_LOC, 4 * CHUNK):
            ce = min(c + 4 * CHUNK, 2 * t_steps * B_LOC)
            nc.sync.dma_start(out=xT_sb[:, c:ce], in_=xT_d.ap()[:, c:ce])
        ones_sb = wpool.tile([1, B_LOC], bf16, name="ones_sb")
        nc.vector.memset(ones_sb, 1.0)

        # HAM warmup: ~4.5us of back-to-back dummy matmuls at kernel start
        # (overlapped with the input DMAs) so the PE clock reaches 2.4 GHz
        # before the recurrence; the per-step gaps are far below the ~3.4us
        # idle window, so it never re-throttles.
        warm_sb = wpool.tile([128, 512], bf16, name="warm_sb")
        nc.vector.memset(warm_sb, 0.0)
        psw = ppool.tile([128, 512], f32, name="psw", tag="psw", bufs=1)
        for _ in range(12):
            nc.tensor.matmul(psw, warm_sb[:, 0:128], warm_sb,
                             start=True, stop=True, skip_group_check=True,
                             tile_position=(0, 0))

        psums = {}
        post_last = [None]

        def proj(t):
            # z for the step is accumulated in TWO independent PSUM tiles
            # (column halves A/B of every group's 256-col range), so the
            # tanh of half A gates on half A's own stop instruction — Tile
            # releases accumulation-group readers only at the group's stop,
            # and a single 256-col group would hold tanh-A until the full
            # last matmul drains (~40ns/step later).
            psA = ppool.tile([128, 128], f32, name="psA", tag="psA")
            psB = ppool.tile([128, 128], f32, name="psB", tag="psB")
            if sim:
                nc.vector.memset(psA, 0.0)
                nc.vector.memset(psB, 0.0)
            psums[t] = (psA, psB)
            for k in range(2):
                o = (t * 2 + k) * B_LOC
                lhsT = xT_sb[:, o:o + B_LOC]
                for half, ps in ((0, psA), (1, psB)):
                    for j in range(4):
                        w0 = H * k + 256 * j + 128 * half
                        mm = nc.tensor.matmul(
                            ps[32 * j:32 * j + 8, :], lhsT,
                            wih_sb[:, w0:w0 + 128],
                            start=(k == 0), stop=False, skip_group_check=True,
                            tile_position=(0, 32 * j))
                        if post_last[0] is not None:
                            # schedule-order only (no semaphore): keeps the
                            # trailing proj MMs AFTER the post ops in Tile's
                            # global order, so the next tanh's PE-tick
                            # target excludes them.
                            add_dep_helper(post_last[0].ins, mm.ins,
                                           sync=False,
                                           reason="post before trailing proj")
                            post_last[0] = None
            for half, ps in ((0, psA), (1, psB)):
                for j in range(4):
                    b0 = 256 * j + 128 * half
                    nc.tensor.matmul(
                        ps[32 * j:32 * j + 8, :], ones_sb,
                        bias_sb[:, b0:b0 + 128],
                        start=False, stop=False, skip_group_check=True,
                        tile_position=(0, 32 * j))

        for t in range(min(LOOKAHEAD, t_steps)):
            proj(t)

        def t_slice(T, f):
            o = 128 * (f // 4) + 32 * (f % 4)
            return T[:, o:o + 8]

        T_prev = None
        for t in range(t_steps):
            psA, psB = psums.pop(t)
            if t > 0:
                for f in range(8):
                    lhsT = t_slice(T_prev, f)
                    for half, ps in ((0, psA), (1, psB)):
                        for j in range(4):
                            w0 = H * f + 256 * j + 128 * half
                            nc.tensor.matmul(
                                ps[32 * j:32 * j + 8, :], lhsT,
                                wt_sb[:, w0:w0 + 128],
                                start=False, stop=(f == 7),
                                skip_group_check=True,
                                tile_position=(0, 32 * j))
            # post split in halves, tanh FIRST (reads PSUM directly, writes
            # bf16), then bf16 block-transpose: tanh-A + tr-A gate rec rounds
            # f=0..3 of the next step; tanh-B/tr-B hide under those rounds.
            S = tpool.tile([128, 256], bf16, name="S", tag="S")
            T = tpool.tile([128, 256], bf16, name="T", tag="T")
            for h, ps in ((0, psA), (1, psB)):
                cs = 128 * h
                a = nc.scalar.activation(out=S[:, cs:cs + 128],
                                         in_=ps, func=Tanh)
                post_last[0] = a
                nc.vector.transpose(out=T[:, cs:cs + 128],
                                    in_=S[:, cs:cs + 128])
            T_prev = T
            # emitted after the post ops so the tanh's semaphore target
            # does not cover these trailing PE instructions (it would
            # over-wait ~0.5us); PE still executes them inside the post gap.
            if t + LOOKAHEAD < t_steps:
                proj(t + LOOKAHEAD)

        psf = ppool.tile([128, 128], f32, name="psf", tag="psf", bufs=1)
        nc.vector.memset(psf, 0.0)
        for f in range(8):
            lhsT = t_slice(T_prev, f)
            nc.tensor.matmul(
                psf[0:8, :], lhsT,
                wlin_sb[:, O * f:O * f + O],
                start=(f == 0), stop=(f == 7), skip_group_check=True,
                tile_position=(0, 0))
        y_sb = tpool.tile([B_LOC, O], f32, name="y_sb", tag="y", bufs=1)
        nc.scalar.copy(out=y_sb, in_=psf[0:B_LOC, :])
        nc.sync.dma_start(out=y_d.ap(), in_=y_sb)

    nc.compile()
    try:
        # All validation precedes any mutation, so a failure here leaves the
        # module intact (just without the sem-traffic optimization).
        _strip_pe_ticks(nc)
    except AssertionError:
        pass
    _module_cache[key] = nc
    return nc


def _host_inputs(x, W_ih, W_hh, b_ih, b_hh, W_lin):
    """Precompute the permuted weight layouts + per-core sharded x."""
    t_steps = x.shape[1]
    wt = np.ascontiguousarray(
        W_hh.T.reshape(4, 8, 32, H).transpose(0, 2, 1, 3).reshape(128, 8 * H)
        .astype(BF16))
    wih = np.ascontiguousarray(
        W_ih.T.reshape(2, 128, H).transpose(1, 0, 2).reshape(128, 2 * H)
        .astype(BF16))
    wlin = np.ascontiguousarray(
        W_lin.T.reshape(4, 8, 32, O).transpose(0, 2, 1, 3).reshape(128, 8 * O)
        .astype(BF16))
    bias1 = np.ascontiguousarray((b_ih + b_hh).reshape(1, H).astype(BF16))

    in_maps = []
    for core in range(NCORES):
        xc = x[core * B_LOC:(core + 1) * B_LOC]  # [8, T, I]
        xT = np.ascontiguousarray(
            xc.transpose(2, 1, 0).reshape(2, 128, t_steps, B_LOC)
            .transpose(1, 2, 0, 3).reshape(128, 2 * t_steps * B_LOC)
            .astype(BF16))
        in_maps.append({"xT": xT, "wt": wt, "wih": wih, "wlin": wlin,
                        "bias1": bias1})
    return in_maps


def kernel(x, W_ih, W_hh, b_ih, b_hh, W_lin, b_lin, _trace=False):
    x = np.asarray(x, np.float32)
    W_ih = np.asarray(W_ih, np.float32)
    W_hh = np.asarray(W_hh, np.float32)
    b_ih = np.asarray(b_ih, np.float32)
    b_hh = np.asarray(b_hh, np.float32)
    W_lin = np.asarray(W_lin, np.float32)
    b_lin = np.asarray(b_lin, np.float32)

    t_steps = x.shape[1]
    nc = _build_module(t_steps)
    in_maps = _host_inputs(x, W_ih, W_hh, b_ih, b_hh, W_lin)

    from concourse.bass_utils import run_bass_kernel_spmd
    res = run_bass_kernel_spmd(nc, in_maps, core_ids=list(range(NCORES)),
                               trace=_trace)
    y = np.concatenate([res.results[c]["y"] for c in range(NCORES)], axis=0)
    if _trace:
        kernel.last_results = res
    return (y + b_lin[None, :]).astype(np.float32)



# revision 29
# speedup vs baseline: 1.4037x; 1.4037x over previous
"""Trainium2 Bass kernel for a single-layer RNN (tanh) + final linear.

Problem: B=64, T=512, I=256, H=1024, O=128 (fp32).
    xp = einsum('bti,hi->tbh', x, W_ih) + b_ih + b_hh
    h_t = tanh(xp_t + h_{t-1} @ W_hh.T)         (T sequential steps)
    y   = h_T @ W_lin.T + b_lin

Sharding: data-parallel over batch, 8 cores x 8 rows each. Each core runs
the full recurrence for its batch shard; no collectives.

Per-core scheme ("T-layout"):
  The recurrence matmul keeps h as the 128x8 stationary operand (batch=8
  output rows per PSUM col-group) and streams W_hh^T quarters through the
  four 32-wide column groups of the PE array concurrently. The PSUM result
  lands batch-major ([32j+b, n] = z[b, 256j+n]). Post chain per step is
  tanh FIRST (ACT reads PSUM directly — faster access — writes bf16 to
  SBUF), then a DVE 32x32 block transpose of the bf16 tanh output into
  T-layout: T[32J+r, 128h+32N+c] = h_new[c, 256J+128h+32N+r], so the
  stationary slice for recurrence k-chunk f=(4h+N) is the contiguous
  T[:, 128h+32N : 128h+32N+8]. The input projection x @ W_ih^T runs in
  bf16 (fp32 moving data costs 4 cycles/row on the PE — 2x LOW/HIGH
  instruction split at half rate) a few steps ahead into the same PSUM
  accumulation group, filling the PE during the tanh/transpose boundary.

All weight-layout permutations are precomputed host-side in numpy.

Post-compile, _strip_pe_ticks removes the per-matmul PE tick-semaphore
increments that no wait targets (keeping exact wait semantics): the hw
drains sem updates at ~34ns each, so the 32 recurrence matmuls per step
otherwise delay the tanh gate by ~190ns of update-queue backlog.
"""

import os
import sys

import ml_dtypes
import numpy as np

BF16 = ml_dtypes.bfloat16

for _p in ("/root/.axon_site", "/root/.axon_site/_ro/trn_rl_repo",
           "/root/.axon_site/_ro/pypackages", "/opt/trn_rl_repo"):
    if os.path.isdir(_p) and _p not in sys.path:
        sys.path.append(_p)

B, I, H, O = 64, 256, 1024, 128
NCORES = 8
B_LOC = B // NCORES  # 8
LOOKAHEAD = 4        # projection runs this many steps ahead of the recurrence

_module_cache = {}


def _strip_pe_ticks(nc):
    """Drop the per-matmul PE tick-sem increments that nothing targets.

    Every InstMatmult carries a +1 update on the PE tick semaphore, and the
    hardware drains these at ~34ns each — so the 32 recurrence matmuls of a
    step delay the tanh's semaphore gate by ~190ns beyond f7's completion
    (the update queue backlog).  Keep only the increments whose tick index
    is the exact target of some wait (plus the last), and remap every wait
    value to its rank among kept ticks.  Since the PE executes in order,
    each wait still releases at the completion of the same instruction as
    before — identical ordering semantics, ~40x less semaphore traffic.
    """
    import concourse.mybir as mybir

    f = nc.m.functions[0]
    insts = []
    for bb in f.blocks:
        insts.extend(bb.instructions)
    tickers = []
    for i in insts:
        if isinstance(i, mybir.InstMatmult) and i.sync_info:
            for u in i.sync_info.on_update:
                if u.update_mode == "sem-inc":
                    tickers.append((i, u.id))
    if not tickers:
        return
    semid = tickers[0][1]
    assert all(s == semid for _, s in tickers)
    waits = []
    for i in insts:
        si = i.sync_info
        if not si:
            continue
        for w in si.on_wait:
            if w.id == semid:
                assert w.wait_mode == "sem-ge-imm" and w.wait_reg is None
                waits.append(w)
    n = len(tickers)
    keep = {n - 1}
    for w in waits:
        assert 1 <= w.wait_value <= n, (w.wait_value, n)
        keep.add(w.wait_value - 1)
    rank = [0] * n
    c = 0
    for idx in range(n):
        if idx in keep:
            c += 1
        rank[idx] = c
    for w in waits:
        w.wait_value = rank[w.wait_value - 1]
    for idx, (i, _) in enumerate(tickers):
        if idx not in keep:
            i.sync_info.on_update = [
                u for u in i.sync_info.on_update
                if not (u.id == semid and u.update_mode == "sem-inc")]


def _build_module(t_steps, sim=False):
    """Trace + compile the Bass module for a given sequence length."""
    key = (t_steps, sim)
    if key in _module_cache:
        return _module_cache[key]

    from contextlib import ExitStack

    import concourse.bacc as bacc
    import concourse.mybir as mybir
    import concourse.tile as tile
    from concourse.tile_rust import add_dep_helper

    f32 = mybir.dt.float32
    bf16 = mybir.dt.bfloat16
    Tanh = mybir.ActivationFunctionType.Tanh

    nc = bacc.Bacc("TRN2", target_bir_lowering=False, debug=False,
                   enable_asserts=False)

    xT_d = nc.dram_tensor("xT", [128, 2 * t_steps * B_LOC], bf16,
                          kind="ExternalInput")
    wt_d = nc.dram_tensor("wt", [128, 8 * H], bf16, kind="ExternalInput")
    wih_d = nc.dram_tensor("wih", [128, 2 * H], bf16, kind="ExternalInput")
    wlin_d = nc.dram_tensor("wlin", [128, 8 * O], bf16, kind="ExternalInput")
    bias_d = nc.dram_tensor("bias1", [1, H], bf16, kind="ExternalInput")
    y_d = nc.dram_tensor("y", [B_LOC, O], f32, kind="ExternalOutput")

    with tile.TileContext(nc) as tc, ExitStack() as ctx:
        wpool = ctx.enter_context(tc.tile_pool(name="weights", bufs=1))
        ppool = ctx.enter_context(tc.tile_pool(name="psum", bufs=LOOKAHEAD + 2,
                                               space="PSUM"))
        tpool = ctx.enter_context(tc.tile_pool(name="tbuf", bufs=2))

        # Small inputs first (proj(0) gates on them), then the 2 MB wt
        # (needed ~2 steps later), then the rest of x lazily; xT is t-major
        # ((t, k, b) columns) and DMA'd in chunks so proj(0) gates on the
        # first 32 steps of x, not the whole 2 MB.
        wih_sb = wpool.tile([128, 2 * H], bf16, name="wih_sb")
        nc.sync.dma_start(out=wih_sb, in_=wih_d.ap())
        bias_sb = wpool.tile([1, H], bf16, name="bias_sb")
        nc.sync.dma_start(out=bias_sb, in_=bias_d.ap())
        xT_sb = wpool.tile([128, 2 * t_steps * B_LOC], bf16, name="xT_sb")
        CHUNK = 32 * 2 * B_LOC
        nc.sync.dma_start(out=xT_sb[:, 0:CHUNK], in_=xT_d.ap()[:, 0:CHUNK])
        wt_sb = wpool.tile([128, 8 * H], bf16, name="wt_sb")
        nc.sync.dma_start(out=wt_sb, in_=wt_d.ap())
        wlin_sb = wpool.tile([128, 8 * O], bf16, name="wlin_sb")
        nc.sync.dma_start(out=wlin_sb, in_=wlin_d.ap())
        for c in range(CHUNK, 2 * t_steps * B_LOC, 4 * CHUNK):
            ce = min(c + 4 * CHUNK, 2 * t_steps * B_LOC)
            nc.sync.dma_start(out=xT_sb[:, c:ce], in_=xT_d.ap()[:, c:ce])
        ones_sb = wpool.tile([1, B_LOC], bf16, name="ones_sb")
        nc.vector.memset(ones_sb, 1.0)

        # HAM warmup: ~4.5us of back-to-back dummy matmuls at kernel start
        # (overlapped with the input DMAs) so the PE clock reaches 2.4 GHz
        # before the recurrence; the per-step gaps are far below the ~3.4us
        # idle window, so it never re-throttles.
        warm_sb = wpool.tile([128, 512], bf16, name="warm_sb")
        nc.vector.memset(warm_sb, 0.0)
        psw = ppool.tile([128, 512], f32, name="psw", tag="psw", bufs=1)
        for _ in range(12):
            nc.tensor.matmul(psw, warm_sb[:, 0:128], warm_sb,
                             start=True, stop=True, skip_group_check=True,
                             tile_position=(0, 0))

        psums = {}
        post_last = [None]

        def proj(t):
            ps = ppool.tile([128, 256], f32, name="ps", tag="ps")
            if sim:
                nc.vector.memset(ps, 0.0)
            psums[t] = ps
            for k in range(2):
                o = (t * 2 + k) * B_LOC
                lhsT = xT_sb[:, o:o + B_LOC]
                for j in range(4):
                    mm = nc.tensor.matmul(
                        ps[32 * j:32 * j + 8, :], lhsT,
                        wih_sb[:, H * k + 256 * j:H * k + 256 * j + 256],
                        start=(k == 0), stop=False, skip_group_check=True,
                        tile_position=(0, 32 * j))
                    if post_last[0] is not None:
                        # schedule-order only (no semaphore): keeps the
                        # trailing proj MMs AFTER the post ops in Tile's
                        # global order, so the next tanh's PE-tick
                        # target excludes them.
                        add_dep_helper(post_last[0].ins, mm.ins, sync=False,
                                       reason="post before trailing proj")
                        post_last[0] = None
            for j in range(4):
                nc.tensor.matmul(
                    ps[32 * j:32 * j + 8, :], ones_sb,
                    bias_sb[:, 256 * j:256 * j + 256],
                    start=False, stop=False, skip_group_check=True,
                    tile_position=(0, 32 * j))

        for t in range(min(LOOKAHEAD, t_steps)):
            proj(t)

        def t_slice(T, f):
            o = 128 * (f // 4) + 32 * (f % 4)
            return T[:, o:o + 8]

        T_prev = None
        for t in range(t_steps):
            ps = psums.pop(t)
            if t > 0:
                for f in range(8):
                    lhsT = t_slice(T_prev, f)
                    for j in range(4):
                        nc.tensor.matmul(
                            ps[32 * j:32 * j + 8, :], lhsT,
                            wt_sb[:, H * f + 256 * j:H * f + 256 * j + 256],
                            start=False, stop=(f == 7), skip_group_check=True,
                            tile_position=(0, 32 * j))
            # post split in halves, tanh FIRST (reads PSUM directly, writes
            # bf16), then bf16 block-transpose: tanh-A + tr-A gate rec rounds
            # f=0..3 of the next step; tanh-B/tr-B hide under those rounds.
            S = tpool.tile([128, 256], bf16, name="S", tag="S")
            T = tpool.tile([128, 256], bf16, name="T", tag="T")
            for h in range(2):
                cs = 128 * h
                a = nc.scalar.activation(out=S[:, cs:cs + 128],
                                         in_=ps[:, cs:cs + 128], func=Tanh)
                post_last[0] = a
                nc.vector.transpose(out=T[:, cs:cs + 128],
                                    in_=S[:, cs:cs + 128])
            T_prev = T
            # emitted after the post ops so the tanh's semaphore target
            # does not cover these trailing PE instructions (it would
            # over-wait ~0.5us); PE still executes them inside the post gap.
            if t + LOOKAHEAD < t_steps:
                proj(t + LOOKAHEAD)

        psf = ppool.tile([128, 128], f32, name="psf", tag="psf", bufs=1)
        nc.vector.memset(psf, 0.0)
        for f in range(8):
            lhsT = t_slice(T_prev, f)
            nc.tensor.matmul(
                psf[0:8, :], lhsT,
                wlin_sb[:, O * f:O * f + O],
                start=(f == 0), stop=(f == 7), skip_group_check=True,
                tile_position=(0, 0))
        y_sb = tpool.tile([B_LOC, O], f32, name="y_sb", tag="y", bufs=1)
        nc.scalar.copy(out=y_sb, in_=psf[0:B_LOC, :])
        nc.sync.dma_start(out=y_d.ap(), in_=y_sb)

    nc.compile()
    try:
        # All validation precedes any mutation, so a failure here leaves the
        # module intact (just without the sem-traffic optimization).
        _strip_pe_ticks(nc)
    except AssertionError:
        pass
    _module_cache[key] = nc
    return nc


def _host_inputs(x, W_ih, W_hh, b_ih, b_hh, W_lin):
    """Precompute the permuted weight layouts + per-core sharded x."""
    t_steps = x.shape[1]
    wt = np.ascontiguousarray(
        W_hh.T.reshape(4, 8, 32, H).transpose(0, 2, 1, 3).reshape(128, 8 * H)
        .astype(BF16))
    wih = np.ascontiguousarray(
        W_ih.T.reshape(2, 128, H).transpose(1, 0, 2).reshape(128, 2 * H)
        .astype(BF16))
    wlin = np.ascontiguousarray(
        W_lin.T.reshape(4, 8, 32, O).transpose(0, 2, 1, 3).reshape(128, 8 * O)
        .astype(BF16))
    bias1 = np.ascontiguousarray((b_ih + b_hh).reshape(1, H).astype(BF16))

    in_maps = []
    for core in range(NCORES):
        xc = x[core * B_LOC:(core + 1) * B_LOC]  # [8, T, I]
        xT = np.ascontiguousarray(
            xc.transpose(2, 1, 0).reshape(2, 128, t_steps, B_LOC)
            .transpose(1, 2, 0, 3).reshape(128, 2 * t_steps * B_LOC)
            .astype(BF16))
        in_maps.append({"xT": xT, "wt": wt, "wih": wih, "wlin": wlin,
                        "bias1": bias1})
    return in_maps


def kernel(x, W_ih, W_hh, b_ih, b_hh, W_lin, b_lin, _trace=False):
    x = np.asarray(x, np.float32)
    W_ih = np.asarray(W_ih, np.float32)
    W_hh = np.asarray(W_hh, np.float32)
    b_ih = np.asarray(b_ih, np.float32)
    b_hh = np.asarray(b_hh, np.float32)
    W_lin = np.asarray(W_lin, np.float32)
    b_lin = np.asarray(b_lin, np.float32)

    t_steps = x.shape[1]
    nc = _build_module(t_steps)
    in_maps = _host_inputs(x, W_ih, W_hh, b_ih, b_hh, W_lin)

    from concourse.bass_utils import run_bass_kernel_spmd
    res = run_bass_kernel_spmd(nc, in_maps, core_ids=list(range(NCORES)),
                               trace=_trace)
    y = np.concatenate([res.results[c]["y"] for c in range(NCORES)], axis=0)
    if _trace:
        kernel.last_results = res
    return (y + b_lin[None, :]).astype(np.float32)

